# revision 18
# baseline (speedup 1.0000x reference)
"""Distributed GAT (AnomalyDAE encoder) kernel for 8 TRN2 NeuronCores.

Reference computation:
    h = leaky_relu(x @ W_dense.T + b_dense, 0.01)          # [N, 128]
    g = h @ W_gat.T                                        # [N, 64]
    a_src = g @ att_src ; a_dst = g @ att_dst              # [N]
    with self-loops appended, per edge (s -> d):
        e = leaky_relu(a_src[s] + a_dst[d], 0.2)
        alpha = segment_softmax(e, by d)
    out[d] = sum_e alpha_e * g[s_e] + b_gat                # [N, 64]

Sharding: nodes split contiguously across 8 cores (6250 each); edges
partitioned by destination core. Per-core nodes are degree-sorted so
128-node tiles have near-uniform degree.

Design (v2):
- Node phase computes a PACKED per-node row [g fp16 x64 | u=e^{a_src} |
  v=e^{0.2 a_src}] (132 B) that is AllGathered in 7 chunks (chunk-major
  gid layout) and locally expanded to a 256 B-stride table `full` for
  the edge gather (dma_gather rows must be 256 B multiples).
- Edge phase: per-dst slots are split across THREE fixed int16 windows
  (L=[0,32K), M=[mid,mid+32K), H=[NTAB-32K,NTAB)). Any gid lies in >=1
  window and the overlap slack lets per-dst slot counts be balanced so
  the per-tile slot grid stays as tight as a single-window grid. Fixed
  windows mean gather calls MERGE across tiles: 3 calls per 7-tile
  group (21 calls total vs ~300), slashing SWDGE descriptor-gen time.
- Edge weight on-chip: w = max(u*p_d, v*q_d) with p=e^{a_dst},
  q=e^{0.2 a_dst} (identity e^{lrelu(x,0.2)} = max(e^x, e^{0.2x});
  softmax computed without max-shift). Pad slots point at a zeroed pad
  row so their weight is exactly 0.
- Weighted sums via in-place fp16 multiply + contiguous binary-tree
  adds (no strided tensor_reduce over the channel axis).
"""

import numpy as np

R = 8            # cores
P = 128          # partitions / tile size
W_ROW = 128      # table row width in fp16 elems (256 B)
PACK = 66        # packed row width: 64 g + u + v
UCOL, VCOL = 64, 65


class Cfg:
    def __init__(self, N, E, IN=512, EMB=128, OUT=64, NCH=7,
                 group_sizes=None, WIN=32768, NW=5):
        assert N % R == 0
        self.N, self.E, self.IN, self.EMB, self.OUT = N, E, IN, EMB, OUT
        self.NL = N // R
        self.NL_pad = ((self.NL + 2 + P - 1) // P) * P
        self.TILES = self.NL_pad // P
        self.NTAB = self.NL_pad * R
        self.NCH = NCH                       # all-gather chunks
        assert self.TILES % self.NCH == 0
        if group_sizes is None:
            group_sizes = [self.TILES // 7] * 7
        assert sum(group_sizes) == self.TILES
        self.group_sizes = group_sizes
        self.group_t0 = np.cumsum([0] + group_sizes[:-1]).tolist()
        self.GROUPS = len(group_sizes)
        self.GTMAX = max(group_sizes)
        self.TPC = self.TILES // self.NCH    # tiles per chunk
        self.CH = self.NL_pad // self.NCH    # rows per chunk per core
        self.WIN = min(WIN, self.NTAB)
        if self.WIN >= self.NTAB:
            self.bases = [0]
        else:
            step = (self.NTAB - self.WIN) / (NW - 1)
            self.bases = sorted({int(round(j * step)) for j in range(NW - 1)}
                                | {self.NTAB - self.WIN})
        self.NWIN = len(self.bases)
        npad = self.NL_pad - self.NL
        base_pads = npad // self.NCH
        self.pads = [base_pads + (1 if c < npad % self.NCH else 0)
                     for c in range(self.NCH)]
        assert min(self.pads) >= 1
        pos = np.arange(self.NL_pad)
        inchunk = pos % self.CH
        self.is_pad = np.zeros(self.NL_pad, bool)
        for c in range(self.NCH):
            self.is_pad[(pos // self.CH == c)
                        & (inchunk >= self.CH - self.pads[c])] = True
        self.positions_real = pos[~self.is_pad]
        assert len(self.positions_real) == self.NL

    def pad_row_in(self, base):
        """A zeroed pad row inside [base, base+WIN): the last row of every
        (chunk, core) subblock is a pad row."""
        s = base + ((self.CH - 1 - base) % self.CH)
        assert base <= s < base + self.WIN and s < self.NTAB
        # verify it is indeed a pad position
        inchunk = s % (R * self.CH) % self.CH
        c = s // (R * self.CH)
        assert inchunk >= self.CH - self.pads[c]
        return s


CFG_REAL = Cfg(N=50000, E=1600000,
               group_sizes=[1, 1, 2, 3, 7, 7, 7, 7, 7, 7])


# --------------------------------------------------------------------------
# host-side preprocessing
# --------------------------------------------------------------------------

def _wrap_idx(lin):
    """dma_gather index layout: linear i -> [i % 16, i // 16], replicated
    across the 8 Q7 core groups -> [128, len/16] int16."""
    assert len(lin) % 16 == 0
    w = lin.reshape(-1, 16).T.astype(np.int16)
    return np.tile(w, (8, 1))


def _build_layout(cfg, src, dst, deg):
    """Multi-window slot grids and per-core index blocks (group metadata
    is core-uniform; index contents are per-core)."""
    N, NL, NL_pad = cfg.N, cfg.NL, cfg.NL_pad
    CH = cfg.CH
    W = cfg.NWIN
    WIN, bases = cfg.WIN, cfg.bases
    pos_of = np.empty(N, dtype=np.int64)
    orders = []
    for r in range(R):
        dloc = deg[r * NL:(r + 1) * NL]
        order = np.argsort(-dloc, kind="stable")
        orders.append(order)
        pos_of[r * NL + order] = cfg.positions_real
    core_of = np.arange(N) // NL
    # chunk-major table: gid = chunk*R*CH + core*CH + pos%CH
    gid_of = (pos_of // CH) * R * CH + core_of * CH + (pos_of % CH)

    slots = []   # per core: (sorted-by-(dst, gid) gids, starts)
    for r in range(R):
        m = (dst >= r * NL) & (dst < (r + 1) * NL)
        s_r = gid_of[src[m]]
        dpos = pos_of[dst[m]]
        order = np.lexsort((s_r, dpos))
        s_s, d_s = s_r[order], dpos[order]
        starts = np.zeros(NL_pad + 1, np.int64)
        np.add.at(starts, d_s + 1, 1)
        starts = np.cumsum(starts)
        slots.append((s_s, starts))

    padrows = [cfg.pad_row_in(b) for b in bases]
    # interval boundaries: lo_bound[j] = first gid only reachable by
    # windows >= j; hi_bound[j] = first gid NOT reachable by windows <= j.
    lo_bound = [0] + [bases[j - 1] + WIN for j in range(1, W)]
    hi_bound = [bases[j + 1] for j in range(W - 1)] + [cfg.NTAB]
    # per-dst we need cnt(gid < x) at every lo/hi boundary
    bounds = sorted(set(lo_bound + hi_bound + [b + WIN for b in bases]))
    bidx = {b: i for i, b in enumerate(bounds)}

    group_meta = []
    idx_blocks = [[] for _ in range(R)]
    for g in range(cfg.GROUPS):
        GT = cfg.group_sizes[g]
        lo_pos = cfg.group_t0[g] * P
        hi_pos = lo_pos + GT * P
        ndst = hi_pos - lo_pos
        cnts = []
        for r in range(R):
            s_s, starts = slots[r]
            cc = np.empty((ndst, len(bounds)), np.int64)
            degs = np.empty(ndst, np.int64)
            for j, d in enumerate(range(lo_pos, hi_pos)):
                seg = s_s[starts[d]:starts[d + 1]]
                degs[j] = len(seg)
                cc[j] = np.searchsorted(seg, bounds)
            cnts.append((cc, degs))

        # minimal window capacities (greedy by right endpoint over all
        # interval constraints, pooled across cores)
        S = [0] * W
        for j2 in range(W):
            for j1 in range(j2 + 1):
                r_need = 0
                for cc, degs in cnts:
                    c_hi = cc[:, bidx[hi_bound[j2]]]
                    c_lo = cc[:, bidx[lo_bound[j1]]]
                    r_need = max(r_need, int((c_hi - c_lo).max()))
                have = sum(S[j1:j2 + 1])
                if r_need > have:
                    S[j2] += r_need - have
        group_meta.append((GT, tuple(S)))

        for r in range(R):
            s_s, starts = slots[r]
            cc, degs = cnts[r]
            G = [np.full((GT * P, S[j]), -1, np.int64) if S[j] else None
                 for j in range(W)]
            for j, d in enumerate(range(lo_pos, hi_pos)):
                seg = s_s[starts[d]:starts[d + 1]]
                dg = int(degs[j])
                if dg == 0:
                    continue
                ptr = 0
                for wj in range(W):
                    if ptr >= dg:
                        break
                    if S[wj] == 0:
                        continue
                    assert seg[ptr] >= bases[wj], (g, r, j, wj)
                    can = int(cc[j, bidx[bases[wj] + WIN]]) - ptr
                    take = min(S[wj], can)
                    # everything that cannot go later must fit now
                    must = int(cc[j, bidx[hi_bound[wj]]]) - ptr
                    assert take >= must, (g, r, j, wj, take, must)
                    if take > 0:
                        G[wj][j, :take] = seg[ptr:ptr + take]
                        ptr += take
                assert ptr == dg, (g, r, j, ptr, dg)
            for wj in range(W):
                if not S[wj]:
                    continue
                Gw = G[wj]
                lin = np.concatenate(
                    [Gw[t * P:(t + 1) * P, :].T.ravel() for t in range(GT)])
                lin[lin < 0] = padrows[wj]
                lin = lin - bases[wj]
                assert lin.min() >= 0 and lin.max() < WIN
                idx_blocks[r].append(_wrap_idx(lin))

    offs = [np.ascontiguousarray(np.concatenate(b, axis=1)) for b in idx_blocks]
    return orders, group_meta, offs


def _prepare(cfg, x, edge_index, W_dense, b_dense, W_gat, att_src, att_dst,
             b_gat):
    import ml_dtypes
    bf16 = ml_dtypes.bfloat16
    N, NL, NL_pad, TILES = cfg.N, cfg.NL, cfg.NL_pad, cfg.TILES
    src = edge_index[0].astype(np.int64)
    dst = edge_index[1].astype(np.int64)
    loops = np.arange(N, dtype=np.int64)
    src = np.concatenate([src, loops])
    dst = np.concatenate([dst, loops])
    deg = np.bincount(dst, minlength=N)

    orders, group_meta, offs = _build_layout(cfg, src, dst, deg)

    wdT = np.ascontiguousarray(W_dense.T)            # [IN, EMB]
    wdT_packed = np.concatenate(
        [wdT[k * P:(k + 1) * P, :] for k in range(cfg.IN // P)], axis=1)
    att = np.concatenate([att_src, att_dst])         # [2*OUT]
    attmat = np.tile(att[None, :], (P, 1)).astype(np.float16)
    bgmatg = np.tile(b_gat[None, :], (P, cfg.GTMAX)).astype(np.float32)
    KC = cfg.IN // P

    in_maps = []
    for r in range(R):
        xp = np.zeros((NL_pad, cfg.IN), dtype=np.float32)
        xp[cfg.positions_real] = x[r * NL + orders[r]]
        xT = np.empty((P, TILES * KC * P), dtype=bf16)
        for t in range(TILES):
            blk = xp[t * P:(t + 1) * P, :].T.astype(bf16)   # [IN, P]
            xT[:, (t * KC) * P:(t + 1) * KC * P] = \
                blk.reshape(KC, P, P).transpose(1, 0, 2).reshape(P, KC * P)
        in_maps.append({
            "xT": xT,
            "wdT": wdT_packed.astype(bf16),
            "bd": b_dense.reshape(cfg.EMB, 1).astype(np.float32),
            "wgT": np.ascontiguousarray(W_gat.T).astype(bf16),
            "attmat": attmat,
            "bgmatg": bgmatg,
            "offs": offs[r],
        })
    return in_maps, orders, group_meta


def _assemble(cfg, results, orders):
    out = np.empty((cfg.N, cfg.OUT), dtype=np.float32)
    for r in range(R):
        o = results[r]["out"][cfg.positions_real]
        out[r * cfg.NL + orders[r]] = o
    return out


# --------------------------------------------------------------------------
# device graph
# --------------------------------------------------------------------------

def _build_graph(cfg, group_meta, tree4d=True):
    import concourse.bass as bass  # noqa: F401
    import concourse.bacc as bacc
    import concourse.mybir as mybir
    import concourse.tile as tile
    from concourse.masks import make_identity

    IN, EMB, OUT = cfg.IN, cfg.EMB, cfg.OUT
    KC = IN // P
    TILES, NL_pad, NTAB = cfg.TILES, cfg.NL_pad, cfg.NTAB
    GTMAX = cfg.GTMAX
    TOTCOLS = sum(GT * sum(S) for (GT, S) in group_meta)
    fp32 = mybir.dt.float32
    b16 = mybir.dt.bfloat16
    h16 = mybir.dt.float16
    i16 = mybir.dt.int16
    AF = mybir.ActivationFunctionType
    OPS = mybir.AluOpType

    nc = bacc.Bacc(None, target_bir_lowering=False, debug=False, num_devices=R,
                   num_swdge_queues=4)

    xT = nc.dram_tensor("xT", [P, TILES * KC * P], b16, kind="ExternalInput")
    wdT = nc.dram_tensor("wdT", [P, KC * EMB], b16, kind="ExternalInput")
    bd = nc.dram_tensor("bd", [EMB, 1], fp32, kind="ExternalInput")
    wgT = nc.dram_tensor("wgT", [EMB, OUT], b16, kind="ExternalInput")
    attmat_in = nc.dram_tensor("attmat", [P, 2 * OUT], h16, kind="ExternalInput")
    bgmatg_in = nc.dram_tensor("bgmatg", [P, GTMAX * OUT], fp32,
                               kind="ExternalInput")
    offs_ext = nc.dram_tensor("offs", [P, 8 * TOTCOLS], i16, kind="ExternalInput")
    out = nc.dram_tensor("out", [NL_pad, OUT], fp32, kind="ExternalOutput")

    with tile.TileContext(nc) as tc:
        with (
            tc.tile_pool(name="dram", bufs=1, space="DRAM") as dram,
            tc.tile_pool(name="const", bufs=1) as cst,
        ):
            shardp = dram.tile([NL_pad, PACK], h16)
            fullp = dram.tile([NTAB, PACK], h16)
            full = dram.tile([NTAB, W_ROW], h16)
            barsrc = dram.tile([1, 64], fp32)
            barout = dram.tile([R, 64], fp32)

            # rendezvous barrier: absorb SPMD core skew before the real
            # AllGather chain so AG0 is not delayed by the slowest core.
            # (collective in/out must be Internal DRAM; content is unused.)
            nc.gpsimd.collective_compute(
                "AllGather", mybir.AluOpType.bypass,
                replica_groups=[list(range(R))],
                ins=[barsrc[:, :].opt()],
                outs=[barout[:, :].opt()],
            )

            identb = cst.tile([P, P], b16)
            make_identity(nc, identb[:])

            wdTs = cst.tile([P, KC * EMB], b16)
            nc.sync.dma_start(out=wdTs[:], in_=wdT[:, :])
            bds = cst.tile([EMB, 1], fp32)
            nc.sync.dma_start(out=bds[:], in_=bd[:, :])
            wgTs = cst.tile([EMB, OUT], b16)
            nc.sync.dma_start(out=wgTs[:], in_=wgT[:, :])
            attmat = cst.tile([P, 2 * OUT], h16)
            nc.sync.dma_start(out=attmat[:], in_=attmat_in[:, :])
            bgmatg = cst.tile([P, GTMAX * OUT], fp32)
            nc.sync.dma_start(out=bgmatg[:], in_=bgmatg_in[:, :])
            adst_all = cst.tile([P, TILES], fp32)
            padst = cst.tile([P, TILES], h16)
            qadst = cst.tile([P, TILES], h16)
            zpad = cst.tile([max(cfg.pads), PACK], h16)
            nc.vector.memset(zpad[:], 0.0)

            # ---------------- node phase ----------------
            with (
                tc.tile_pool(name="npsum_h", bufs=2, space="PSUM") as ps_h,
                tc.tile_pool(name="npsum_m", bufs=2, space="PSUM") as ps_m,
                tc.tile_pool(name="nsb", bufs=3) as nsb,
            ):
                for t in range(TILES):
                    xTs = nsb.tile([P, KC * P], b16, tag="xTs")
                    nc.sync.dma_start(
                        out=xTs[:], in_=xT[:, t * KC * P:(t + 1) * KC * P])
                    hTp = ps_h.tile([EMB, P], fp32, tag="hT")
                    for k in range(KC):
                        nc.tensor.matmul(out=hTp[:],
                                         lhsT=wdTs[:, k * EMB:(k + 1) * EMB],
                                         rhs=xTs[:, k * P:(k + 1) * P],
                                         start=(k == 0), stop=(k == KC - 1))
                    u = nsb.tile([EMB, P], fp32, tag="u")
                    nc.scalar.activation(u[:], hTp[:], AF.Identity,
                                         bias=bds[:, :1])
                    hT = nsb.tile([EMB, P], b16, tag="hT_sb")
                    nc.vector.scalar_tensor_tensor(
                        out=hT[:], in0=u[:], scalar=0.01, in1=u[:],
                        op0=OPS.mult, op1=OPS.max)
                    gTp = ps_m.tile([OUT, P], fp32, tag="gTp")
                    nc.tensor.matmul(out=gTp[:], lhsT=wgTs[:], rhs=hT[:],
                                     start=True, stop=True)
                    stg = nsb.tile([OUT, P], b16, tag="stg")
                    nc.vector.tensor_copy(stg[:], gTp[:])
                    # transpose gT -> packed table g block
                    ttp = ps_m.tile([P, OUT], b16, tag="ttp")
                    nc.tensor.transpose(out=ttp[:], in_=stg[:],
                                        identity=identb[:OUT, :OUT])
                    tabs = nsb.tile([P, PACK], h16, tag="tabs")
                    nc.scalar.activation(tabs[:, 0:OUT], ttp[:], AF.Copy)
                    # a_src/a_dst as per-partition row-dots with att columns
                    gw = nsb.tile([P, 2 * OUT], fp32, tag="gw")
                    nc.vector.tensor_tensor(
                        out=gw[:, 0:OUT], in0=tabs[:, 0:OUT],
                        in1=attmat[:, 0:OUT], op=OPS.mult)
                    nc.vector.tensor_tensor(
                        out=gw[:, OUT:2 * OUT], in0=tabs[:, 0:OUT],
                        in1=attmat[:, OUT:2 * OUT], op=OPS.mult)
                    asrcv = nsb.tile([P, 1], fp32, tag="asrcv")
                    nc.vector.tensor_reduce(
                        out=asrcv[:], in_=gw[:, 0:OUT], op=OPS.add,
                        axis=mybir.AxisListType.X)
                    nc.vector.tensor_reduce(
                        out=adst_all[:, t:t + 1],
                        in_=gw[:, OUT:2 * OUT], op=OPS.add,
                        axis=mybir.AxisListType.X)
                    # u = e^{a_src}, v = e^{0.2 a_src}
                    nc.scalar.activation(tabs[:, UCOL:UCOL + 1], asrcv[:],
                                         AF.Exp)
                    nc.scalar.activation(tabs[:, VCOL:VCOL + 1], asrcv[:],
                                         AF.Exp, scale=0.2)
                    nc.sync.dma_start(
                        out=shardp[t * P:(t + 1) * P, :], in_=tabs[:])

                    # chunk boundary: zero pad rows; AllGather; expand
                    if (t + 1) % cfg.TPC == 0:
                        c = t // cfg.TPC
                        pc = cfg.pads[c]
                        lo = (c + 1) * cfg.CH - pc
                        nc.sync.dma_start(
                            out=shardp[lo:lo + pc, :], in_=zpad[:pc, :])
                        nc.gpsimd.collective_compute(
                            "AllGather", mybir.AluOpType.bypass,
                            replica_groups=[list(range(R))],
                            ins=[shardp[c * cfg.CH:(c + 1) * cfg.CH, :].opt()],
                            outs=[fullp[c * R * cfg.CH:
                                        (c + 1) * R * cfg.CH, :].opt()],
                        )
                        # expand packed 132 B rows -> 256 B-stride table
                        nc.sync.dma_start(
                            out=full[c * R * cfg.CH:(c + 1) * R * cfg.CH,
                                     0:PACK],
                            in_=fullp[c * R * cfg.CH:(c + 1) * R * cfg.CH, :])

                nc.scalar.activation(padst[:], adst_all[:], AF.Exp)
                nc.scalar.activation(qadst[:], adst_all[:], AF.Exp, scale=0.2)

            # ---------------- edge phase ----------------
            qi = 0
            allidx = cst.tile([P, 8 * TOTCOLS], i16)
            nc.sync.dma_start(out=allidx[:], in_=offs_ext[:, :])
            with tc.tile_pool(name="esb", bufs=2) as esb:
                cum = 0
                NW = cfg.NWIN
                for g in range(cfg.GROUPS):
                    GT, widths = group_meta[g]
                    t0 = cfg.group_t0[g]
                    S = sum(widths) * GT
                    blkoff = np.cumsum([0] + [Dw * GT for Dw in widths])
                    rows = esb.tile([P, S * W_ROW], h16, tag="rows")
                    for wi in range(NW):
                        Dw = widths[wi]
                        if Dw == 0:
                            continue
                        ncols = Dw * GT
                        c0 = int(blkoff[wi])
                        base = cfg.bases[wi]
                        nc.gpsimd.dma_gather(
                            out_ap=rows[:, c0 * W_ROW:(c0 + ncols) * W_ROW]
                                .rearrange("p (j e) -> p j e", e=W_ROW),
                            in_ap=full[base:base + cfg.WIN, :],
                            idxs_ap=allidx[:, 8 * cum:8 * (cum + ncols)],
                            num_idxs=ncols * P, num_idxs_reg=ncols * P,
                            elem_size=W_ROW,
                            single_packet=False,
                            queue_num=0,
                        )
                        qi += 1
                        cum += ncols

                    # per-window-block 4D view [P, GT, Dw, W_ROW]
                    def blk4(wi, Dw):
                        c0 = int(blkoff[wi])
                        return rows[:, c0 * W_ROW:(c0 + Dw * GT) * W_ROW] \
                            .rearrange("p (t d e) -> p t d e", d=Dw, e=W_ROW)

                    ps = padst[:, t0:t0 + GT]
                    qs = qadst[:, t0:t0 + GT]
                    w = esb.tile([P, S], h16, tag="w")
                    tmp = esb.tile([P, S], h16, tag="tmp")
                    for wi in range(NW):
                        Dw = widths[wi]
                        if Dw == 0:
                            continue
                        c0 = int(blkoff[wi])
                        v4 = blk4(wi, Dw)
                        ub = v4[:, :, :, UCOL]
                        vb = v4[:, :, :, VCOL]
                        wv = w[:, c0:c0 + Dw * GT].rearrange(
                            "p (t d) -> p t d", d=Dw)
                        tv = tmp[:, c0:c0 + Dw * GT].rearrange(
                            "p (t d) -> p t d", d=Dw)
                        nc.vector.tensor_tensor(
                            out=tv, in0=vb, in1=qs.to_broadcast([P, GT, Dw]),
                            op=OPS.mult)
                        nc.vector.tensor_tensor(
                            out=wv, in0=ub, in1=ps.to_broadcast([P, GT, Dw]),
                            op=OPS.mult)
                        nc.vector.tensor_tensor(
                            out=wv, in0=wv, in1=tv, op=OPS.max)

                    dn = esb.tile([P, GT], fp32, tag="dn")
                    dtmp = esb.tile([P, GT], fp32, tag="dtmp")
                    first = True
                    for wi in range(NW):
                        Dw = widths[wi]
                        if Dw == 0:
                            continue
                        c0 = int(blkoff[wi])
                        wv = w[:, c0:c0 + Dw * GT].rearrange(
                            "p (t d) -> p t d", d=Dw)
                        nc.vector.tensor_reduce(
                            out=(dn[:] if first else dtmp[:]), in_=wv,
                            op=OPS.add, axis=mybir.AxisListType.X)
                        if not first:
                            nc.vector.tensor_tensor(
                                out=dn[:], in0=dn[:], in1=dtmp[:], op=OPS.add)
                        first = False
                    rden = esb.tile([P, GT], fp32, tag="rden")
                    nc.vector.reciprocal(rden[:], dn[:])
                    rden16 = esb.tile([P, GT], h16, tag="rden16")
                    nc.vector.tensor_copy(rden16[:], rden[:])

                    # in-place weighted g
                    gv = rows[:].rearrange("p (n e) -> p n e", e=W_ROW)[:, :, 0:OUT]
                    nc.vector.tensor_tensor(
                        out=gv, in0=gv, in1=w[:].to_broadcast([P, S, OUT]),
                        op=OPS.mult)

                    # binary-tree segment sums (contiguous adds)
                    for wi in range(NW):
                        Dw = widths[wi]
                        if Dw == 0:
                            continue
                        v4g = blk4(wi, Dw)[:, :, :, 0:OUT]
                        if tree4d:
                            cur = Dw
                            while cur > 1:
                                h = cur // 2
                                nc.vector.tensor_tensor(
                                    out=v4g[:, :, 0:h, :],
                                    in0=v4g[:, :, 0:h, :],
                                    in1=v4g[:, :, cur - h:cur, :],
                                    op=OPS.add)
                                cur -= h
                        else:
                            for t in range(GT):
                                v3 = v4g[:, t]
                                cur = Dw
                                while cur > 1:
                                    h = cur // 2
                                    nc.vector.tensor_tensor(
                                        out=v3[:, 0:h, :], in0=v3[:, 0:h, :],
                                        in1=v3[:, cur - h:cur, :], op=OPS.add)
                                    cur -= h

                    heads = [blk4(wi, widths[wi])[:, :, 0, 0:OUT]
                             for wi in range(NW) if widths[wi] > 0]
                    onum = esb.tile([P, GT * OUT], h16, tag="onum")
                    onv = onum[:].rearrange("p (t c) -> p t c", c=OUT)
                    if len(heads) == 1:
                        nc.vector.tensor_copy(onv, heads[0])
                    else:
                        nc.vector.tensor_tensor(
                            out=onv, in0=heads[0], in1=heads[1], op=OPS.add)
                        for hd in heads[2:]:
                            nc.vector.tensor_tensor(
                                out=onv, in0=onv, in1=hd, op=OPS.add)
                    outv = esb.tile([P, GT * OUT], fp32, tag="outv")
                    nc.vector.tensor_tensor(
                        out=outv[:].rearrange("p (t c) -> p t c", c=OUT),
                        in0=onv,
                        in1=rden16[:].to_broadcast([P, GT, OUT]),
                        op=OPS.mult)
                    outfg = esb.tile([P, GT * OUT], fp32, tag="outf")
                    nc.vector.tensor_tensor(
                        out=outfg[:], in0=outv[:], in1=bgmatg[:, :GT * OUT],
                        op=OPS.add)
                    nc.sync.dma_start(
                        out=out[t0 * P:(t0 + GT) * P, :]
                            .rearrange("(t p) c -> p t c", p=P),
                        in_=outfg[:].rearrange("p (t c) -> p t c", c=OUT))
    nc.finalize()
    return nc


# --------------------------------------------------------------------------
# entry points
# --------------------------------------------------------------------------

def run(inputs, cfg=CFG_REAL, trace=False):
    from concourse.bass_utils import run_bass_kernel_spmd
    in_maps, orders, group_meta = _prepare(cfg, **inputs)
    nc = _build_graph(cfg, group_meta)
    res = run_bass_kernel_spmd(nc, in_maps, core_ids=list(range(R)),
                               trace=trace)
    out = _assemble(cfg, res.results, orders)
    return out, res


def kernel(**inputs):
    inputs = {k: np.asarray(v) for k, v in inputs.items()}
    out, _ = run(inputs, CFG_REAL, trace=False)
    return out


# revision 25
# speedup vs baseline: 2.7367x; 2.7367x over previous
"""Distributed GAT (AnomalyDAE encoder) kernel for 8 TRN2 NeuronCores.

Reference computation:
    h = leaky_relu(x @ W_dense.T + b_dense, 0.01)          # [N, 128]
    g = h @ W_gat.T                                        # [N, 64]
    a_src = g @ att_src ; a_dst = g @ att_dst              # [N]
    with self-loops appended, per edge (s -> d):
        e = leaky_relu(a_src[s] + a_dst[d], 0.2)
        alpha = segment_softmax(e, by d)
    out[d] = sum_e alpha_e * g[s_e] + b_gat                # [N, 64]

Sharding: nodes split contiguously across 8 cores (6250 each); edges
partitioned by destination core. Per-core nodes are degree-sorted so
128-node tiles have near-uniform degree.

Design (v2):
- Node phase computes a PACKED per-node row [g fp16 x64 | u=e^{a_src} |
  v=e^{0.2 a_src}] (132 B) that is AllGathered in 7 chunks (chunk-major
  gid layout) and locally expanded to a 256 B-stride table `full` for
  the edge gather (dma_gather rows must be 256 B multiples).
- Edge phase: per-dst slots are split across THREE fixed int16 windows
  (L=[0,32K), M=[mid,mid+32K), H=[NTAB-32K,NTAB)). Any gid lies in >=1
  window and the overlap slack lets per-dst slot counts be balanced so
  the per-tile slot grid stays as tight as a single-window grid. Fixed
  windows mean gather calls MERGE across tiles: 3 calls per 7-tile
  group (21 calls total vs ~300), slashing SWDGE descriptor-gen time.
- Edge weight on-chip: w = max(u*p_d, v*q_d) with p=e^{a_dst},
  q=e^{0.2 a_dst} (identity e^{lrelu(x,0.2)} = max(e^x, e^{0.2x});
  softmax computed without max-shift). Pad slots point at a zeroed pad
  row so their weight is exactly 0.
- Weighted sums via in-place fp16 multiply + contiguous binary-tree
  adds (no strided tensor_reduce over the channel axis).
"""

import numpy as np

R = 8            # cores
P = 128          # partitions / tile size
W_ROW = 128      # table row width in fp16 elems (256 B)
UCOL, VCOL = 64, 65
SUBCALL = 32     # gather sub-call width in slot columns (32*128 = 4096 idxs)


class Cfg:
    def __init__(self, N, E, IN=512, EMB=128, OUT=64, NCH=7,
                 group_sizes=None, WIN=32768, NW=5):
        assert N % R == 0
        self.N, self.E, self.IN, self.EMB, self.OUT = N, E, IN, EMB, OUT
        self.NL = N // R
        self.NL_pad = ((self.NL + 2 + P - 1) // P) * P
        self.TILES = self.NL_pad // P
        self.NTAB = self.NL_pad * R
        self.NCH = NCH                       # all-gather chunks
        assert self.TILES % self.NCH == 0
        if group_sizes is None:
            group_sizes = [self.TILES // 7] * 7
        assert sum(group_sizes) == self.TILES
        self.group_sizes = group_sizes
        self.group_t0 = np.cumsum([0] + group_sizes[:-1]).tolist()
        self.GROUPS = len(group_sizes)
        self.GTMAX = max(group_sizes)
        self.TPC = self.TILES // self.NCH    # tiles per chunk
        self.CH = self.NL_pad // self.NCH    # rows per chunk per core
        self.WIN = min(WIN, self.NTAB)
        if self.WIN >= self.NTAB:
            self.bases = [0]
        else:
            step = (self.NTAB - self.WIN) / (NW - 1)
            self.bases = sorted({int(round(j * step)) for j in range(NW - 1)}
                                | {self.NTAB - self.WIN})
        self.NWIN = len(self.bases)
        npad = self.NL_pad - self.NL
        base_pads = npad // self.NCH
        self.pads = [base_pads + (1 if c < npad % self.NCH else 0)
                     for c in range(self.NCH)]
        assert min(self.pads) >= 1
        pos = np.arange(self.NL_pad)
        inchunk = pos % self.CH
        self.is_pad = np.zeros(self.NL_pad, bool)
        for c in range(self.NCH):
            self.is_pad[(pos // self.CH == c)
                        & (inchunk >= self.CH - self.pads[c])] = True
        self.positions_real = pos[~self.is_pad]
        assert len(self.positions_real) == self.NL

    def pad_row_in(self, base):
        """A zeroed pad row inside [base, base+WIN): the last row of every
        (chunk, core) subblock is a pad row."""
        s = base + ((self.CH - 1 - base) % self.CH)
        assert base <= s < base + self.WIN and s < self.NTAB
        # verify it is indeed a pad position
        inchunk = s % (R * self.CH) % self.CH
        c = s // (R * self.CH)
        assert inchunk >= self.CH - self.pads[c]
        return s


CFG_REAL = Cfg(N=50000, E=1600000,
               group_sizes=[1, 1, 2, 3, 7, 7, 7, 7, 7, 7])


# --------------------------------------------------------------------------
# host-side preprocessing
# --------------------------------------------------------------------------

def _wrap_idx(lin):
    """dma_gather index layout: linear i -> [i % 16, i // 16], replicated
    across the 8 Q7 core groups -> [128, len/16] int16."""
    assert len(lin) % 16 == 0
    w = lin.reshape(-1, 16).T.astype(np.int16)
    return np.tile(w, (8, 1))


def _build_layout(cfg, src, dst, deg):
    """Multi-window slot grids and per-core index blocks (group metadata
    is core-uniform; index contents are per-core)."""
    N, NL, NL_pad = cfg.N, cfg.NL, cfg.NL_pad
    CH = cfg.CH
    W = cfg.NWIN
    WIN, bases = cfg.WIN, cfg.bases
    pos_of = np.empty(N, dtype=np.int64)
    orders = []
    for r in range(R):
        dloc = deg[r * NL:(r + 1) * NL]
        order = np.argsort(-dloc, kind="stable")
        orders.append(order)
        pos_of[r * NL + order] = cfg.positions_real
    core_of = np.arange(N) // NL
    # chunk-major table: gid = chunk*R*CH + core*CH + pos%CH
    gid_of = (pos_of // CH) * R * CH + core_of * CH + (pos_of % CH)

    slots = []   # per core: (sorted-by-(dst, gid) gids, starts)
    for r in range(R):
        m = (dst >= r * NL) & (dst < (r + 1) * NL)
        s_r = gid_of[src[m]]
        dpos = pos_of[dst[m]]
        order = np.lexsort((s_r, dpos))
        s_s, d_s = s_r[order], dpos[order]
        starts = np.zeros(NL_pad + 1, np.int64)
        np.add.at(starts, d_s + 1, 1)
        starts = np.cumsum(starts)
        slots.append((s_s, starts))

    padrows = [cfg.pad_row_in(b) for b in bases]
    # interval boundaries: lo_bound[j] = first gid only reachable by
    # windows >= j; hi_bound[j] = first gid NOT reachable by windows <= j.
    lo_bound = [0] + [bases[j - 1] + WIN for j in range(1, W)]
    hi_bound = [bases[j + 1] for j in range(W - 1)] + [cfg.NTAB]
    # per-dst we need cnt(gid < x) at every lo/hi boundary
    bounds = sorted(set(lo_bound + hi_bound + [b + WIN for b in bases]))
    bidx = {b: i for i, b in enumerate(bounds)}

    group_meta = []
    idx_blocks = [[] for _ in range(R)]
    for g in range(cfg.GROUPS):
        GT = cfg.group_sizes[g]
        lo_pos = cfg.group_t0[g] * P
        hi_pos = lo_pos + GT * P
        ndst = hi_pos - lo_pos
        cnts = []
        for r in range(R):
            s_s, starts = slots[r]
            cc = np.empty((ndst, len(bounds)), np.int64)
            degs = np.empty(ndst, np.int64)
            for j, d in enumerate(range(lo_pos, hi_pos)):
                seg = s_s[starts[d]:starts[d + 1]]
                degs[j] = len(seg)
                cc[j] = np.searchsorted(seg, bounds)
            cnts.append((cc, degs))

        # minimal window capacities (greedy by right endpoint over all
        # interval constraints, pooled across cores)
        S = [0] * W
        for j2 in range(W):
            for j1 in range(j2 + 1):
                r_need = 0
                for cc, degs in cnts:
                    c_hi = cc[:, bidx[hi_bound[j2]]]
                    c_lo = cc[:, bidx[lo_bound[j1]]]
                    r_need = max(r_need, int((c_hi - c_lo).max()))
                have = sum(S[j1:j2 + 1])
                if r_need > have:
                    S[j2] += r_need - have
        group_meta.append((GT, tuple(S)))

        for r in range(R):
            s_s, starts = slots[r]
            cc, degs = cnts[r]
            G = [np.full((GT * P, S[j]), -1, np.int64) if S[j] else None
                 for j in range(W)]
            for j, d in enumerate(range(lo_pos, hi_pos)):
                seg = s_s[starts[d]:starts[d + 1]]
                dg = int(degs[j])
                if dg == 0:
                    continue
                ptr = 0
                for wj in range(W):
                    if ptr >= dg:
                        break
                    if S[wj] == 0:
                        continue
                    assert seg[ptr] >= bases[wj], (g, r, j, wj)
                    can = int(cc[j, bidx[bases[wj] + WIN]]) - ptr
                    take = min(S[wj], can)
                    # everything that cannot go later must fit now
                    must = int(cc[j, bidx[hi_bound[wj]]]) - ptr
                    assert take >= must, (g, r, j, wj, take, must)
                    if take > 0:
                        G[wj][j, :take] = seg[ptr:ptr + take]
                        ptr += take
                assert ptr == dg, (g, r, j, ptr, dg)
            for wj in range(W):
                if not S[wj]:
                    continue
                Gw = G[wj]
                lin = np.concatenate(
                    [Gw[t * P:(t + 1) * P, :].T.ravel() for t in range(GT)])
                lin[lin < 0] = padrows[wj]
                lin = lin - bases[wj]
                assert lin.min() >= 0 and lin.max() < WIN
                idx_blocks[r].append(_wrap_idx(lin))

    offs = [np.ascontiguousarray(np.concatenate(b, axis=1)) for b in idx_blocks]
    return orders, group_meta, offs


def _prepare(cfg, x, edge_index, W_dense, b_dense, W_gat, att_src, att_dst,
             b_gat):
    import ml_dtypes
    bf16 = ml_dtypes.bfloat16
    N, NL, NL_pad, TILES = cfg.N, cfg.NL, cfg.NL_pad, cfg.TILES
    src = edge_index[0].astype(np.int64)
    dst = edge_index[1].astype(np.int64)
    loops = np.arange(N, dtype=np.int64)
    src = np.concatenate([src, loops])
    dst = np.concatenate([dst, loops])
    deg = np.bincount(dst, minlength=N)

    orders, group_meta, offs = _build_layout(cfg, src, dst, deg)

    wdT = np.ascontiguousarray(W_dense.T)            # [IN, EMB]
    wdT_packed = np.concatenate(
        [wdT[k * P:(k + 1) * P, :] for k in range(cfg.IN // P)], axis=1)
    att = np.concatenate([att_src, att_dst])         # [2*OUT]
    attmat = np.tile(att[None, :], (P, 1)).astype(np.float16)
    bgmatg = np.tile(b_gat[None, :], (P, cfg.GTMAX)).astype(np.float32)
    KC = cfg.IN // P

    in_maps = []
    for r in range(R):
        xp = np.zeros((NL_pad, cfg.IN), dtype=np.float32)
        xp[cfg.positions_real] = x[r * NL + orders[r]]
        xT = np.empty((P, TILES * KC * P), dtype=bf16)
        for t in range(TILES):
            blk = xp[t * P:(t + 1) * P, :].T.astype(bf16)   # [IN, P]
            xT[:, (t * KC) * P:(t + 1) * KC * P] = \
                blk.reshape(KC, P, P).transpose(1, 0, 2).reshape(P, KC * P)
        in_maps.append({
            "xT": xT,
            "wdT": wdT_packed.astype(bf16),
            "bd": b_dense.reshape(cfg.EMB, 1).astype(np.float32),
            "wgT": np.ascontiguousarray(W_gat.T).astype(bf16),
            "attmat": attmat,
            "bgmatg": bgmatg,
            "offs": offs[r],
        })
    return in_maps, orders, group_meta


def _assemble(cfg, results, orders):
    out = np.empty((cfg.N, cfg.OUT), dtype=np.float32)
    for r in range(R):
        o = results[r]["out"][cfg.positions_real]
        out[r * cfg.NL + orders[r]] = o
    return out


# --------------------------------------------------------------------------
# device graph
# --------------------------------------------------------------------------

def _build_graph(cfg, group_meta, queues=4):
    import concourse.bass as bass  # noqa: F401
    import concourse.bacc as bacc
    import concourse.mybir as mybir
    import concourse.tile as tile
    from concourse.masks import make_identity

    IN, EMB, OUT = cfg.IN, cfg.EMB, cfg.OUT
    KC = IN // P
    TILES, NL_pad, NTAB = cfg.TILES, cfg.NL_pad, cfg.NTAB
    GTMAX = cfg.GTMAX
    TOTCOLS = sum(GT * sum(S) for (GT, S) in group_meta)
    fp32 = mybir.dt.float32
    b16 = mybir.dt.bfloat16
    h16 = mybir.dt.float16
    i16 = mybir.dt.int16
    AF = mybir.ActivationFunctionType
    OPS = mybir.AluOpType

    nc = bacc.Bacc(None, target_bir_lowering=False, debug=False, num_devices=R,
                   num_swdge_queues=4)

    xT = nc.dram_tensor("xT", [P, TILES * KC * P], b16, kind="ExternalInput")
    wdT = nc.dram_tensor("wdT", [P, KC * EMB], b16, kind="ExternalInput")
    bd = nc.dram_tensor("bd", [EMB, 1], fp32, kind="ExternalInput")
    wgT = nc.dram_tensor("wgT", [EMB, OUT], b16, kind="ExternalInput")
    attmat_in = nc.dram_tensor("attmat", [P, 2 * OUT], h16, kind="ExternalInput")
    bgmatg_in = nc.dram_tensor("bgmatg", [P, GTMAX * OUT], fp32,
                               kind="ExternalInput")
    offs_ext = nc.dram_tensor("offs", [P, 8 * TOTCOLS], i16, kind="ExternalInput")
    out = nc.dram_tensor("out", [NL_pad, OUT], fp32, kind="ExternalOutput")

    with tile.TileContext(nc) as tc:
        with (
            tc.tile_pool(name="dram", bufs=1, space="DRAM") as dram,
            tc.tile_pool(name="const", bufs=1) as cst,
        ):
            shard = dram.tile([NL_pad, W_ROW], h16)
            full = dram.tile([NTAB, W_ROW], h16)

            identb = cst.tile([P, P], b16)
            make_identity(nc, identb[:])

            wdTs = cst.tile([P, KC * EMB], b16)
            nc.sync.dma_start(out=wdTs[:], in_=wdT[:, :])
            bds = cst.tile([EMB, 1], fp32)
            nc.sync.dma_start(out=bds[:], in_=bd[:, :])
            wgTs = cst.tile([EMB, OUT], b16)
            nc.sync.dma_start(out=wgTs[:], in_=wgT[:, :])
            attmat = cst.tile([P, 2 * OUT], h16)
            nc.sync.dma_start(out=attmat[:], in_=attmat_in[:, :])
            bgmatg = cst.tile([P, GTMAX * OUT], fp32)
            nc.sync.dma_start(out=bgmatg[:], in_=bgmatg_in[:, :])
            adst_all = cst.tile([P, TILES], fp32)
            padst = cst.tile([P, TILES], h16)
            zpad = cst.tile([max(cfg.pads), W_ROW], h16)
            nc.vector.memset(zpad[:], 0.0)

            # ---------------- node phase ----------------
            with (
                tc.tile_pool(name="npsum_h", bufs=2, space="PSUM") as ps_h,
                tc.tile_pool(name="npsum_m", bufs=2, space="PSUM") as ps_m,
                tc.tile_pool(name="nsb", bufs=3) as nsb,
            ):
                for t in range(TILES):
                    xTs = nsb.tile([P, KC * P], b16, tag="xTs")
                    nc.sync.dma_start(
                        out=xTs[:], in_=xT[:, t * KC * P:(t + 1) * KC * P])
                    hTp = ps_h.tile([EMB, P], fp32, tag="hT")
                    for k in range(KC):
                        nc.tensor.matmul(out=hTp[:],
                                         lhsT=wdTs[:, k * EMB:(k + 1) * EMB],
                                         rhs=xTs[:, k * P:(k + 1) * P],
                                         start=(k == 0), stop=(k == KC - 1))
                    u = nsb.tile([EMB, P], fp32, tag="u")
                    nc.scalar.activation(u[:], hTp[:], AF.Identity,
                                         bias=bds[:, :1])
                    hT = nsb.tile([EMB, P], b16, tag="hT_sb")
                    nc.vector.scalar_tensor_tensor(
                        out=hT[:], in0=u[:], scalar=0.01, in1=u[:],
                        op0=OPS.mult, op1=OPS.max)
                    gTp = ps_m.tile([OUT, P], fp32, tag="gTp")
                    nc.tensor.matmul(out=gTp[:], lhsT=wgTs[:], rhs=hT[:],
                                     start=True, stop=True)
                    stg = nsb.tile([OUT, P], b16, tag="stg")
                    nc.vector.tensor_copy(stg[:], gTp[:])
                    # transpose gT -> table g block
                    ttp = ps_m.tile([P, OUT], b16, tag="ttp")
                    nc.tensor.transpose(out=ttp[:], in_=stg[:],
                                        identity=identb[:OUT, :OUT])
                    tabs = nsb.tile([P, W_ROW], h16, tag="tabs")
                    nc.scalar.activation(tabs[:, 0:OUT], ttp[:], AF.Copy)
                    nc.vector.memset(tabs[:, VCOL + 1:], 0.0)
                    # a_src/a_dst as per-partition row-dots with att columns
                    gw = nsb.tile([P, 2 * OUT], fp32, tag="gw")
                    nc.vector.tensor_tensor(
                        out=gw[:, 0:OUT], in0=tabs[:, 0:OUT],
                        in1=attmat[:, 0:OUT], op=OPS.mult)
                    nc.vector.tensor_tensor(
                        out=gw[:, OUT:2 * OUT], in0=tabs[:, 0:OUT],
                        in1=attmat[:, OUT:2 * OUT], op=OPS.mult)
                    asrcv = nsb.tile([P, 1], fp32, tag="asrcv")
                    nc.vector.tensor_reduce(
                        out=asrcv[:], in_=gw[:, 0:OUT], op=OPS.add,
                        axis=mybir.AxisListType.X)
                    nc.vector.tensor_reduce(
                        out=adst_all[:, t:t + 1],
                        in_=gw[:, OUT:2 * OUT], op=OPS.add,
                        axis=mybir.AxisListType.X)
                    # u = e^{a_src}, v = e^{0.2 a_src}
                    nc.scalar.activation(tabs[:, UCOL:UCOL + 1], asrcv[:],
                                         AF.Exp)
                    nc.scalar.activation(tabs[:, VCOL:VCOL + 1], asrcv[:],
                                         AF.Exp, scale=0.2)
                    nc.sync.dma_start(
                        out=shard[t * P:(t + 1) * P, :], in_=tabs[:])

                    # chunk boundary: zero pad rows; AllGather
                    if (t + 1) % cfg.TPC == 0:
                        c = t // cfg.TPC
                        pc = cfg.pads[c]
                        lo = (c + 1) * cfg.CH - pc
                        nc.sync.dma_start(
                            out=shard[lo:lo + pc, :], in_=zpad[:pc, :])
                        nc.gpsimd.collective_compute(
                            "AllGather", mybir.AluOpType.bypass,
                            replica_groups=[list(range(R))],
                            ins=[shard[c * cfg.CH:(c + 1) * cfg.CH, :].opt()],
                            outs=[full[c * R * cfg.CH:
                                       (c + 1) * R * cfg.CH, :].opt()],
                        )

                # w = e^{lrelu(as+ad,0.2)} ~ (per-dst scale) max(u*p', v)
                # with u=e^{as}, v=e^{0.2 as}, p'=e^{0.8 ad}
                nc.scalar.activation(padst[:], adst_all[:], AF.Exp, scale=0.8)

            # ---------------- edge phase ----------------
            qi = 0
            with (
                tc.tile_pool(name="esb2", bufs=2) as esb2,
                tc.tile_pool(name="esb1", bufs=1) as esb1,
            ):
                cum = 0
                NW = cfg.NWIN
                for g in range(cfg.GROUPS):
                    GT, widths = group_meta[g]
                    t0 = cfg.group_t0[g]
                    S = sum(widths) * GT
                    blkoff = np.cumsum([0] + [Dw * GT for Dw in widths])
                    rows = esb2.tile([P, S * W_ROW], h16, tag="rows")
                    gidx = esb2.tile([P, 8 * S], i16, tag="gidx")
                    nc.sync.dma_start(
                        out=gidx[:], in_=offs_ext[:, 8 * cum:8 * (cum + S)])
                    icum = 0
                    for wi in range(NW):
                        Dw = widths[wi]
                        if Dw == 0:
                            continue
                        ncols = Dw * GT
                        c0 = int(blkoff[wi])
                        base = cfg.bases[wi]
                        # split into sub-calls (queue-parallel, bounded DGE)
                        for s0 in range(0, ncols, SUBCALL):
                            sn = min(SUBCALL, ncols - s0)
                            nc.gpsimd.dma_gather(
                                out_ap=rows[:, (c0 + s0) * W_ROW:
                                            (c0 + s0 + sn) * W_ROW]
                                    .rearrange("p (j e) -> p j e", e=W_ROW),
                                in_ap=full[base:base + cfg.WIN, :],
                                idxs_ap=gidx[:, 8 * (icum + s0):
                                             8 * (icum + s0 + sn)],
                                num_idxs=sn * P, num_idxs_reg=sn * P,
                                elem_size=W_ROW,
                                single_packet=False,
                                queue_num=(qi % 4) if queues > 1 else 0,
                            )
                            qi += 1
                        icum += ncols
                    cum += S

                    # per-window-block 4D view [P, GT, Dw, W_ROW]
                    def blk4(wi, Dw):
                        c0 = int(blkoff[wi])
                        return rows[:, c0 * W_ROW:(c0 + Dw * GT) * W_ROW] \
                            .rearrange("p (t d e) -> p t d e", d=Dw, e=W_ROW)

                    ps = padst[:, t0:t0 + GT]
                    w = esb1.tile([P, S], h16, tag="w")
                    for wi in range(NW):
                        Dw = widths[wi]
                        if Dw == 0:
                            continue
                        c0 = int(blkoff[wi])
                        v4 = blk4(wi, Dw)
                        ub = v4[:, :, :, UCOL]
                        vb = v4[:, :, :, VCOL]
                        wv = w[:, c0:c0 + Dw * GT].rearrange(
                            "p (t d) -> p t d", d=Dw)
                        # w = max(u * p', v)
                        nc.vector.tensor_tensor(
                            out=wv, in0=ub, in1=ps.to_broadcast([P, GT, Dw]),
                            op=OPS.mult)
                        nc.vector.tensor_tensor(
                            out=wv, in0=wv, in1=vb, op=OPS.max)

                    dn = esb1.tile([P, GT], fp32, tag="dn")
                    dtmp = esb1.tile([P, GT], fp32, tag="dtmp")
                    first = True
                    for wi in range(NW):
                        Dw = widths[wi]
                        if Dw == 0:
                            continue
                        c0 = int(blkoff[wi])
                        wv = w[:, c0:c0 + Dw * GT].rearrange(
                            "p (t d) -> p t d", d=Dw)
                        nc.vector.tensor_reduce(
                            out=(dn[:] if first else dtmp[:]), in_=wv,
                            op=OPS.add, axis=mybir.AxisListType.X)
                        if not first:
                            nc.vector.tensor_tensor(
                                out=dn[:], in0=dn[:], in1=dtmp[:], op=OPS.add)
                        first = False
                    rden = esb1.tile([P, GT], fp32, tag="rden")
                    nc.vector.reciprocal(rden[:], dn[:])

                    # weighted g into contiguous gsc (fp16)
                    gsc = esb1.tile([P, S * OUT], h16, tag="gsc")
                    gv = rows[:].rearrange("p (n e) -> p n e",
                                           e=W_ROW)[:, :, 0:OUT]
                    nc.vector.tensor_tensor(
                        out=gsc[:].rearrange("p (n c) -> p n c", c=OUT),
                        in0=gv, in1=w[:].to_broadcast([P, S, OUT]),
                        op=OPS.mult)

                    # binary-tree segment sums (contiguous 2x-mode adds)
                    for wi in range(NW):
                        Dw = widths[wi]
                        if Dw == 0:
                            continue
                        c0 = int(blkoff[wi])
                        gb = gsc[:, c0 * OUT:(c0 + Dw * GT) * OUT] \
                            .rearrange("p (t x) -> p t x", x=Dw * OUT)
                        cur = Dw
                        while cur > 1:
                            h = cur // 2
                            nc.vector.tensor_tensor(
                                out=gb[:, :, 0:h * OUT],
                                in0=gb[:, :, 0:h * OUT],
                                in1=gb[:, :, (cur - h) * OUT:cur * OUT],
                                op=OPS.add)
                            cur -= h

                    def head(wi):
                        c0 = int(blkoff[wi])
                        Dw = widths[wi]
                        return gsc[:, c0 * OUT:(c0 + Dw * GT) * OUT] \
                            .rearrange("p (t x) -> p t x",
                                       x=Dw * OUT)[:, :, 0:OUT]
                    heads = [head(wi) for wi in range(NW) if widths[wi] > 0]
                    onum = esb1.tile([P, GT * OUT], h16, tag="onum")
                    onv = onum[:].rearrange("p (t c) -> p t c", c=OUT)
                    if len(heads) == 1:
                        nc.vector.tensor_copy(onv, heads[0])
                    else:
                        nc.vector.tensor_tensor(
                            out=onv, in0=heads[0], in1=heads[1], op=OPS.add)
                        for hd in heads[2:]:
                            nc.vector.tensor_tensor(
                                out=onv, in0=onv, in1=hd, op=OPS.add)
                    outv = esb1.tile([P, GT * OUT], fp32, tag="outv")
                    nc.vector.tensor_tensor(
                        out=outv[:].rearrange("p (t c) -> p t c", c=OUT),
                        in0=onv,
                        in1=rden[:].to_broadcast([P, GT, OUT]),
                        op=OPS.mult)
                    outfg = esb1.tile([P, GT * OUT], fp32, tag="outf")
                    nc.vector.tensor_tensor(
                        out=outfg[:], in0=outv[:], in1=bgmatg[:, :GT * OUT],
                        op=OPS.add)
                    nc.sync.dma_start(
                        out=out[t0 * P:(t0 + GT) * P, :]
                            .rearrange("(t p) c -> p t c", p=P),
                        in_=outfg[:].rearrange("p (t c) -> p t c", c=OUT))
    nc.finalize()
    return nc


# --------------------------------------------------------------------------
# entry points
# --------------------------------------------------------------------------

def run(inputs, cfg=CFG_REAL, trace=False, queues=4):
    from concourse.bass_utils import run_bass_kernel_spmd
    in_maps, orders, group_meta = _prepare(cfg, **inputs)
    nc = _build_graph(cfg, group_meta, queues=queues)
    res = run_bass_kernel_spmd(nc, in_maps, core_ids=list(range(R)),
                               trace=trace)
    out = _assemble(cfg, res.results, orders)
    return out, res


def kernel(**inputs):
    inputs = {k: np.asarray(v) for k, v in inputs.items()}
    out, _ = run(inputs, CFG_REAL, trace=False)
    return out
